# revision 49
# baseline (speedup 1.0000x reference)
"""Trainium2 Bass kernel for nn_MoEBlock (8-expert top-2 MoE + shared expert).

Strategy (v2): expert-parallel sparse MoE across 8 NeuronCores.
 - Gate via AllGather: each core computes fp32 logits for its 512 home
   tokens, AllGathers [4096, 8] (131KB), then a batched softmax/top-2
   (3D-AP vector ops + per-core one-hot "my expert" selector) yields
   mask/gate for all tokens in a few instructions.
 - Slot map via matmul prefix sums (cumsum^T = mask^T @ triu).
 - Dispatch: ONE multi-column indirect DMA scatters (token_idx, gate)
   pairs into slot space; routed X rows are then gathered directly from
   DRAM by token index (no Xcomp roundtrip, no DMA transposes - X tiles
   are transposed on the PE array).
 - FFN: W1 resident in SBUF; W2 streamed per group; exact Gelu+b1 on ACT.
   FFN2 output is gate-scaled in slot space and row-scattered straight
   into the pre-zeroed token-order partial buffer.
 - ReduceScatter (bf16) -> 512 home tokens per core; shared expert FFN is
   split: first half at kernel start (covers routing latency), second
   half + FFN2 + top-2-weighted b2 bias (K=9 matmul) at the end
   (covers the ReduceScatter).
 - Host: concatenates the 8 home slices; host work is layout/dtype only.
"""

import numpy as np
import ml_dtypes
from contextlib import ExitStack

import concourse.bass as bass
import concourse.tile as tile
from concourse import bacc, mybir
from concourse.bass import IndirectOffsetOnAxis
from concourse.bass_utils import run_bass_kernel_spmd

# Register the axon NTFF profiling hook if the image's antenv lacks it
# (needed only for trace=True; harmless otherwise).
try:
    from antenv.axon_hooks import get_axon_ntff_profile_hook  # noqa: F401
except ImportError:
    try:
        import sys
        import types
        import antenv
        from trn_agent_boot.trn_boot import _ntff_profile_via_ctypes
        _mod = types.ModuleType("antenv.axon_hooks")
        _mod._hook = _ntff_profile_via_ctypes("/opt/axon/libaxon_pjrt.so")
        _mod.get_axon_ntff_profile_hook = lambda: _mod._hook
        _mod.set_axon_ntff_profile_hook = lambda h: setattr(_mod, "_hook", h)
        sys.modules["antenv.axon_hooks"] = _mod
        antenv.axon_hooks = _mod
    except Exception:
        pass

BF16 = ml_dtypes.bfloat16
T, D, INNER, E = 4096, 1024, 4096, 8
N_CORES = 8
HOME = T // N_CORES            # 512
CAP = 1280                     # padded per-expert token capacity
NTILE = CAP // 128             # 10
KT = D // 128                  # 8 k-tiles of model dim
MT = INNER // 128              # 32 i-tiles of inner dim
NJ = T // 128                  # 32 token tiles
GROUPS = [(0, 4), (4, 4), (8, 2)]   # (slot-tile start, n slot-tiles)
SH_SPLIT = 6                   # shared FFN1 m-tiles emitted before the slotmap
NPAIR = 8                      # parallel pair-scatter arrays

_CACHE: dict = {}


def _build_nc(gelu_mode: str = "hw", debug: bool = False):
    dt = mybir.dt
    f32, bf, u32 = dt.float32, dt.bfloat16, dt.uint32
    AF = mybir.ActivationFunctionType
    OP = mybir.AluOpType
    AX = mybir.AxisListType

    nc = bacc.Bacc("TRN2", target_bir_lowering=False, debug=False,
                   num_devices=N_CORES)

    def inp(name, shape, dtype):
        return nc.dram_tensor(name, shape, dtype, kind="ExternalInput")

    Xbf_d = inp("Xbf", [T, D], bf)
    XT32_d = inp("XT32", [128, KT * T], f32)
    XhTb_d = inp("XhTbf", [128, KT * HOME], bf)
    WgT_d = inp("WgT", [128, KT * E], f32)
    W1_d = inp("W1e", [128, KT * INNER], bf)
    W2_d = inp("W2e", [128, MT * D], bf)
    b1_d = inp("b1e", [128, MT], f32)
    sW1_d = inp("sW1e", [128, MT * KT * 128], bf)
    sW2_d = inp("sW2e", [128, MT * D], bf)
    sb1_d = inp("sb1e", [128, MT], f32)
    b2p_d = inp("b2p9", [9, D], bf)
    myhot_d = inp("myhot", [128, E], f32)
    tokf_d = inp("tokf", [128, NJ], f32)
    utri_d = inp("utri", [128, 128], f32)
    sut_d = inp("sutri32", [32, 32], f32)
    id_d = inp("id128", [128, 128], f32)
    idb_d = inp("idbf", [128, 128], bf)
    out_d = nc.dram_tensor("out", [HOME, D], f32, kind="ExternalOutput")
    if debug:
        dbg_scores = nc.dram_tensor("dbg_scores", [128, NJ * E], f32, kind="ExternalOutput")
        dbg_mask = nc.dram_tensor("dbg_mask", [128, NJ], f32, kind="ExternalOutput")
        dbg_gate = nc.dram_tensor("dbg_gate", [128, NJ], f32, kind="ExternalOutput")
        dbg_slot = nc.dram_tensor("dbg_slot", [128, NJ], f32, kind="ExternalOutput")
        dbg_tokg = nc.dram_tensor("dbg_tokg", [128, NTILE * 2], f32, kind="ExternalOutput")
        dbg_partial = nc.dram_tensor("dbg_partial", [T, D], bf, kind="ExternalOutput")

    with tile.TileContext(nc) as tc, ExitStack() as ctx:
        const = ctx.enter_context(tc.tile_pool(name="const", bufs=1))
        persist = ctx.enter_context(tc.tile_pool(name="persist", bufs=1))
        jtp = ctx.enter_context(tc.tile_pool(name="jtp", bufs=2))
        xhp = ctx.enter_context(tc.tile_pool(name="xhp", bufs=1))
        xtp = ctx.enter_context(tc.tile_pool(name="xtp", bufs=2))
        htp = ctx.enter_context(tc.tile_pool(name="htp", bufs=32))
        shp = ctx.enter_context(tc.tile_pool(name="shp", bufs=1))
        stream = ctx.enter_context(tc.tile_pool(name="stream", bufs=6))
        ypool = ctx.enter_context(tc.tile_pool(name="ypool", bufs=6))
        dram = ctx.enter_context(tc.tile_pool(name="dram", bufs=1, space="DRAM"))
        pph = ctx.enter_context(tc.tile_pool(name="pph", bufs=2, space="PSUM"))
        ppy = ctx.enter_context(tc.tile_pool(name="ppy", bufs=4, space="PSUM"))
        ppt = ctx.enter_context(tc.tile_pool(name="ppt", bufs=2, space="PSUM"))

        # ---- DRAM intermediates ----
        # (tok_idx, gate) per slot — split into NPAIR arrays so the SWDGE
        # scatters don't serialize on write-write tile hazards
        pairs = [dram.tile([CAP + 128, 2], f32, name=f"pairs{k}")
                 for k in range(NPAIR)]
        partial = dram.tile([T + 128, D], bf)      # token-order partial (+dump)
        rs_out = dram.tile([HOME, D], bf)

        # ---- constants / resident weights ----
        WgTsb = const.tile([128, KT * E], f32)
        nc.sync.dma_start(WgTsb[:], WgT_d.ap())
        b1sb = const.tile([128, MT], f32)
        nc.sync.dma_start(b1sb[:], b1_d.ap())
        sb1sb = const.tile([128, MT], f32)
        nc.sync.dma_start(sb1sb[:], sb1_d.ap())
        b2psb = const.tile([9, D], bf)
        nc.sync.dma_start(b2psb[:], b2p_d.ap())
        myhot = const.tile([128, E], f32)
        nc.sync.dma_start(myhot[:], myhot_d.ap())
        tokf = const.tile([128, NJ], f32)
        nc.sync.dma_start(tokf[:], tokf_d.ap())
        utrisb = const.tile([128, 128], f32)
        nc.sync.dma_start(utrisb[:], utri_d.ap())
        sutsb = const.tile([32, 32], f32)
        nc.sync.dma_start(sutsb[:], sut_d.ap())
        idsb = const.tile([128, 128], f32)
        nc.sync.dma_start(idsb[:], id_d.ap())
        idbf = const.tile([128, 128], bf)
        nc.sync.dma_start(idbf[:], idb_d.ap())
        ones_sb = const.tile([1, 128], f32)
        nc.vector.memset(ones_sb[:], 1.0)

        # init pairs arrays, rows [0:CAP): tok=T (dump), gate=0
        painit = const.tile([128, 2 * NTILE], f32)
        nc.vector.memset(painit[:].rearrange("p (g t) -> p g t", t=2)[:, :, 0:1],
                         float(T))
        nc.vector.memset(painit[:].rearrange("p (g t) -> p g t", t=2)[:, :, 1:2],
                         0.0)
        for k in range(NPAIR):
            nc.sync.dma_start(
                pairs[k][0:CAP, :].rearrange("(p g) t -> p (g t)", p=128),
                painit[:])

        # sim-only gelu surrogate: x*sigmoid(1.702x) (sim lacks Gelu/Erf)
        if gelu_mode == "sig":
            b1s_sb = const.tile([128, MT], f32)
            nc.vector.tensor_scalar_mul(b1s_sb[:], b1sb[:], 1.702)
            sb1s_sb = const.tile([128, MT], f32)
            nc.vector.tensor_scalar_mul(sb1s_sb[:], sb1sb[:], 1.702)

        def emit_gelu(ht_out, ph, bias_col, bias_s_col):
            if gelu_mode == "sig":
                sg = jtp.tile([128, ph.shape[-1]], f32, tag="gsg", bufs=2)
                nc.scalar.activation(sg[:], ph[:], AF.Sigmoid,
                                     bias=bias_s_col, scale=1.702)
                xb = jtp.tile([128, ph.shape[-1]], f32, tag="gxb", bufs=2)
                nc.vector.tensor_scalar_add(xb[:], ph[:], bias_col)
                nc.vector.tensor_tensor(ht_out, xb[:], sg[:], op=OP.mult)
            else:
                nc.scalar.activation(ht_out, ph[:], AF.Gelu, bias=bias_col)

        # ======== phase 1+2: replicated gate (fp32 X^T streamed, per-core
        # ROTATED token order: home tokens first) interleaved with the
        # shared-expert FFN1 so the PE never starves while streams land.
        scores = persist.tile([128, NJ, E], f32)
        # home-token X^T bf16 (shared FFN rhs; resident until shared FFN2)
        xhbf = xhp.tile([128, KT * HOME], bf)
        nc.sync.dma_start(xhbf[:], XhTb_d.ap())
        shT = []

        def emit_shared_ffn1(m):
            sw1t = stream.tile([128, KT * 128], bf, tag="sw1", bufs=6,
                               name=f"sw1t{m}")
            nc.scalar.dma_start(sw1t[:], sW1_d.ap()[:, m * 1024:(m + 1) * 1024])
            ph = pph.tile([128, HOME], f32, tag="ph")
            for a in range(KT):
                nc.tensor.matmul(ph[:], lhsT=sw1t[:, a * 128:(a + 1) * 128],
                                 rhs=xhbf[:, a * HOME:(a + 1) * HOME],
                                 start=(a == 0), stop=(a == KT - 1))
            ht = shp.tile([128, HOME], bf, tag=f"sht{m}", name=f"sht{m}")
            emit_gelu(ht[:], ph, sb1sb[:, m:m + 1],
                      sb1s_sb[:, m:m + 1] if gelu_mode == "sig" else None)
            shT.append(ht)

        for jg in range(8):
            xw = []
            for a in range(KT):
                t_ = stream.tile([128, 512], f32, tag="xh", bufs=3,
                                 name=f"xw{jg}_{a}")
                nc.sync.dma_start(t_[:], XT32_d.ap()[:, a * T + jg * 512:
                                                     a * T + (jg + 1) * 512])
                xw.append(t_)
            pgs = [ppy.tile([128, E], f32, tag="py", name=f"pg{jg}_{jj}")
                   for jj in range(4)]
            for a in range(KT):
                for jj in range(4):
                    nc.tensor.matmul(
                        pgs[jj][:], lhsT=xw[a][:, jj * 128:(jj + 1) * 128],
                        rhs=WgTsb[:, a * E:(a + 1) * E],
                        start=(a == 0), stop=(a == KT - 1))
            for jj in range(4):
                nc.vector.tensor_copy(scores[:, jg * 4 + jj, :], pgs[jj][:])
            for m in range(4 * jg, 4 * jg + 4):
                emit_shared_ffn1(m)
        glocal = scores[:, 0:4, :]

        # partial zero-fill + resident shared W2 (hoistable; no collective
        # at the front anymore, so plain emission order is fine)
        sW2sb = const.tile([128, MT * D], bf)
        nc.sync.dma_start(sW2sb[:], sW2_d.ap())
        zsb = const.tile([128, D], bf)
        nc.vector.memset(zsb[:], 0.0)
        for j in range(NJ):
            nc.sync.dma_start(partial[j * 128:(j + 1) * 128, :], zsb[:])

        # ======== phase 3: batched routing ========
        m1 = persist.tile([128, NJ], f32)
        nc.vector.tensor_reduce(m1[:], scores[:], axis=AX.X, op=OP.max)
        sub = jtp.tile([128, NJ, E], f32, tag="sub", bufs=1)
        nc.vector.tensor_tensor(sub[:], scores[:],
                                m1[:].unsqueeze(2).broadcast_to((128, NJ, E)),
                                op=OP.subtract)
        et = jtp.tile([128, NJ, E], f32, tag="et", bufs=1)
        nc.scalar.activation(et[:], sub[:], AF.Exp)
        ssum = persist.tile([128, NJ], f32)
        nc.vector.reduce_sum(ssum[:], et[:], axis=AX.X)
        rcp = persist.tile([128, NJ], f32)
        nc.vector.reciprocal(rcp[:], ssum[:])
        tmy = jtp.tile([128, NJ, E], f32, tag="tmp3", bufs=1)
        nc.vector.tensor_tensor(tmy[:], scores[:],
                                myhot[:].unsqueeze(1).broadcast_to((128, NJ, E)),
                                op=OP.mult)
        myl = persist.tile([128, NJ], f32)
        nc.vector.reduce_sum(myl[:], tmy[:], axis=AX.X)
        gtb = jtp.tile([128, NJ, E], f32, tag="tmp3", bufs=1)
        nc.vector.tensor_tensor(gtb[:], scores[:],
                                myl[:].unsqueeze(2).broadcast_to((128, NJ, E)),
                                op=OP.is_gt)
        cnt = persist.tile([128, NJ], f32)
        nc.vector.reduce_sum(cnt[:], gtb[:], axis=AX.X)
        mask_c = persist.tile([128, NJ], f32)
        nc.vector.tensor_scalar(mask_c[:], cnt[:], 1.0, None, op0=OP.is_le)
        tme = jtp.tile([128, NJ, E], f32, tag="tmp3", bufs=1)
        nc.vector.tensor_tensor(tme[:], et[:],
                                myhot[:].unsqueeze(1).broadcast_to((128, NJ, E)),
                                op=OP.mult)
        mye = persist.tile([128, NJ], f32)
        nc.vector.reduce_sum(mye[:], tme[:], axis=AX.X)
        gate_c = persist.tile([128, NJ], f32)
        nc.vector.tensor_tensor(gate_c[:], mye[:], rcp[:], op=OP.mult)
        nc.vector.tensor_tensor(gate_c[:], gate_c[:], mask_c[:], op=OP.mult)

        # ---- slot map (matmul prefix sums) ----
        pcs = ppt.tile([32, 128], f32, tag="pt")
        nc.tensor.matmul(pcs[:], lhsT=mask_c[:], rhs=utrisb[:],
                         start=True, stop=True)      # inclusive cumsum^T
        csT = persist.tile([32, 128], f32)
        nc.vector.tensor_copy(csT[:], pcs[:])
        pBr = ppt.tile([1, 32], f32, tag="pt")
        nc.tensor.matmul(pBr[:], lhsT=csT[:, 127:128], rhs=sutsb[:],
                         start=True, stop=True)      # exclusive block prefix
        Brow = persist.tile([1, 32], f32)
        nc.vector.tensor_copy(Brow[:], pBr[:])
        pslot = ppt.tile([128, 32], f32, tag="pt")
        nc.tensor.matmul(pslot[:], lhsT=csT[:], rhs=idsb[0:32, 0:32],
                         start=True, stop=False)     # transpose back
        nc.tensor.matmul(pslot[:], lhsT=ones_sb[:], rhs=Brow[:],
                         start=False, stop=True)     # + B[j] broadcast
        excl = persist.tile([128, 32], f32)
        nc.vector.tensor_tensor(excl[:], pslot[:], mask_c[:], op=OP.subtract)
        d1 = persist.tile([128, 32], f32)
        nc.vector.tensor_tensor(d1[:], excl[:], mask_c[:], op=OP.mult)
        d2 = persist.tile([128, 32], f32)
        nc.vector.tensor_scalar(d2[:], mask_c[:], -float(CAP + T),
                                float(CAP + T), op0=OP.mult, op1=OP.add)
        slotf2 = persist.tile([128, 32], f32)
        nc.vector.tensor_tensor(slotf2[:], d1[:], d2[:], op=OP.add)
        slotu2 = persist.tile([128, 32], u32)
        nc.vector.tensor_copy(slotu2[:], slotf2[:])

        # ---- (tok, gate) pair scatter into slot space ----
        # NOTE: multi-column offset APs work in CoreSim but NOT on HW —
        # scatter one 128-token column at a time ([128,1] offsets), round-
        # robin across NPAIR arrays so the per-array write-write completion
        # waits overlap each other on the gpsimd queue.
        pairT = persist.tile([128, NJ, 2], f32)
        nc.vector.tensor_copy(pairT[:, :, 0], tokf[:])
        nc.vector.tensor_copy(pairT[:, :, 1], gate_c[:])
        # NOTE: multi-column offset APs work in CoreSim but NOT on HW —
        # scatter one 128-token column at a time ([128,1] offsets).
        for j in range(NJ):
            nc.gpsimd.indirect_dma_start(
                pairs[j % NPAIR][:],
                IndirectOffsetOnAxis(ap=slotu2[:, j:j + 1], axis=0),
                pairT[:, j, :], None, bounds_check=CAP + 127,
                oob_is_err=False)

        # flat reload (row p*NTILE+g = slot; slot-tile g := {s : s%NTILE==g})
        # then merge the NPAIR arrays: tok=min (init T), gate=sum (init 0)
        tokgs = []
        for k in range(NPAIR):
            tg = persist.tile([128, NTILE, 2], f32, name=f"tokg{k}")
            nc.scalar.dma_start(
                tg[:], pairs[k][0:CAP, :].rearrange("(p g) t -> p g t", p=128))
            tokgs.append(tg)
        tokg = persist.tile([128, NTILE, 2], f32)
        nc.vector.tensor_tensor(tokg[:, :, 0], tokgs[0][:, :, 0],
                                tokgs[1][:, :, 0], op=OP.min)
        nc.vector.tensor_tensor(tokg[:, :, 1], tokgs[0][:, :, 1],
                                tokgs[1][:, :, 1], op=OP.add)
        for k in range(2, NPAIR):
            nc.vector.tensor_tensor(tokg[:, :, 0], tokg[:, :, 0],
                                    tokgs[k][:, :, 0], op=OP.min)
            nc.vector.tensor_tensor(tokg[:, :, 1], tokg[:, :, 1],
                                    tokgs[k][:, :, 1], op=OP.add)
        toku = persist.tile([128, NTILE], u32)
        nc.vector.tensor_copy(toku[:], tokg[:, :, 0])
        Gslot = persist.tile([128, NTILE], f32)
        nc.vector.tensor_copy(Gslot[:], tokg[:, :, 1])

        # home combine (all 8 experts + ones col) for the b2/sb2 bias matmul
        m1h = jtp.tile([128, 4], f32, tag="m1h", bufs=1)
        nc.vector.tensor_reduce(m1h[:], glocal[:], axis=AX.X, op=OP.max)
        subh_t = jtp.tile([128, NJ, E], f32, tag="sub", bufs=1, name="subh_t")
        subh = subh_t[:, 0:4, :]
        nc.vector.tensor_tensor(subh[:], glocal[:],
                                m1h[:].unsqueeze(2).broadcast_to((128, 4, E)),
                                op=OP.subtract)
        eth_t = jtp.tile([128, NJ, E], f32, tag="et", bufs=1, name="eth_t")
        eth = eth_t[:, 0:4, :]
        nc.scalar.activation(eth[:], subh[:], AF.Exp)
        ssumh = jtp.tile([128, 4], f32, tag="ssumh", bufs=1)
        nc.vector.reduce_sum(ssumh[:], eth[:], axis=AX.X)
        rcph = jtp.tile([128, 4], f32, tag="rcph", bufs=1)
        nc.vector.reciprocal(rcph[:], ssumh[:])
        gth_t = jtp.tile([128, NJ, E], f32, tag="tmp3", bufs=1, name="gth_t")
        gth = gth_t[:].rearrange("p (x a) e -> p x a e", a=8)
        nc.vector.tensor_tensor(
            gth[:],
            glocal[:].unsqueeze(2).broadcast_to((128, 4, E, E)),
            glocal[:].unsqueeze(3).broadcast_to((128, 4, E, E)), op=OP.is_gt)
        cnth = jtp.tile([128, 4, E], f32, tag="cnth", bufs=1)
        nc.vector.tensor_reduce(cnth[:], gth[:], axis=AX.X, op=OP.add)
        mask8 = jtp.tile([128, 4, E], f32, tag="mask8", bufs=1)
        nc.vector.tensor_scalar(mask8[:], cnth[:], 1.0, None, op0=OP.is_le)
        w8 = jtp.tile([128, 4, E], f32, tag="w8", bufs=1)
        nc.vector.tensor_tensor(w8[:], eth[:], mask8[:], op=OP.mult)
        comb9 = persist.tile([128, 4, 9], f32)
        nc.vector.tensor_tensor(
            comb9[:, :, 0:E], w8[:],
            rcph[:].unsqueeze(2).broadcast_to((128, 4, E)), op=OP.mult)
        nc.vector.memset(comb9[:, :, E:E + 1], 1.0)
        combT = persist.tile([9, 512], bf)
        for jj in range(4):
            pcT = ppt.tile([9, 128], f32, tag="pt")
            nc.tensor.transpose(pcT[:], comb9[:, jj, :], idsb[:])
            nc.vector.tensor_copy(combT[0:9, jj * 128:(jj + 1) * 128], pcT[:])


        if debug:
            nc.sync.dma_start(
                dbg_scores.ap(),
                scores[:].rearrange("p j e -> p (j e)"))
            nc.sync.dma_start(dbg_mask.ap(), mask_c[:])
            nc.sync.dma_start(dbg_gate.ap(), gate_c[:])
            nc.sync.dma_start(dbg_slot.ap(), slotf2[:])
            nc.sync.dma_start(
                dbg_tokg.ap(), tokg[:].rearrange("p g t -> p (g t)"))

        # ======== phase 4: routed expert FFN over slot groups ========
        for gi, (st0, nt) in enumerate(GROUPS):
            W = nt * 128
            # gather routed X rows for this group (direct from DRAM)
            xgs = []
            for st in range(nt):
                s = st0 + st
                xg = stream.tile([128, D], bf, tag="xg", bufs=2,
                                 name=f"xg{s}")
                nc.gpsimd.indirect_dma_start(
                    xg[:], None, Xbf_d.ap(),
                    IndirectOffsetOnAxis(ap=toku[:, s:s + 1], axis=0),
                    bounds_check=T - 1, oob_is_err=False)
                xgs.append(xg)
            # PE transpose -> xT[a] = [d-part, slot]
            xT = [xtp.tile([128, 512], bf, tag=f"xt{a}", bufs=1, name=f"xT{gi}_{a}")
                  for a in range(KT)]
            for st in range(nt):
                for a in range(KT):
                    ptr = ppt.tile([128, 128], bf, tag="pt")
                    nc.tensor.transpose(
                        ptr[:], xgs[st][:, a * 128:(a + 1) * 128], idbf[:])
                    nc.vector.tensor_copy(
                        xT[a][:, st * 128:(st + 1) * 128], ptr[:])
            # FFN1 (W1 streamed, m-contiguous layout) + gelu
            hT = []
            for m in range(MT):
                w1t = stream.tile([128, KT * 128], bf, tag="sw1", bufs=6,
                                  name=f"w1t{gi}_{m}")
                nc.scalar.dma_start(
                    w1t[:], W1_d.ap()[:, m * 1024:(m + 1) * 1024])
                ph = pph.tile([128, 512], f32, tag="ph")
                for a in range(KT):
                    nc.tensor.matmul(
                        ph[:, 0:W],
                        lhsT=w1t[:, a * 128:(a + 1) * 128],
                        rhs=xT[a][:, 0:W], start=(a == 0), stop=(a == KT - 1))
                ht = htp.tile([128, 512], bf, tag="ht", name=f"ht{gi}_{m}")
                emit_gelu(ht[:, 0:W], ph[:, 0:W], b1sb[:, m:m + 1],
                          b1s_sb[:, m:m + 1] if gelu_mode == "sig" else None)
                hT.append(ht)
            # FFN2 (W2 streamed) + gate scale + scatter to partial
            ysb = [ypool.tile([128, D], bf, tag="ysb", bufs=4, name=f"ysb{gi}_{st}")
                   for st in range(nt)]
            for dh in range(2):
                pys = [ppy.tile([128, 512], f32, tag="py", name=f"pys{tt}")
                       for tt in range(nt)]
                for m in range(MT):
                    w2t = stream.tile([128, 512], bf, tag="w2", bufs=3,
                                      name=f"w2t{gi}_{dh}_{m}")
                    nc.sync.dma_start(
                        w2t[:], W2_d.ap()[:, m * D + dh * 512:
                                          m * D + dh * 512 + 512])
                    for tt in range(nt):
                        nc.tensor.matmul(
                            pys[tt][:], lhsT=hT[m][:, tt * 128:(tt + 1) * 128],
                            rhs=w2t[:], start=(m == 0), stop=(m == MT - 1))
                for tt in range(nt):
                    s = st0 + tt
                    nc.scalar.activation(
                        ysb[tt][:, dh * 512:dh * 512 + 512], pys[tt][:],
                        AF.Copy, scale=Gslot[:, s:s + 1])
            for tt in range(nt):
                s = st0 + tt
                nc.gpsimd.indirect_dma_start(
                    partial[:], IndirectOffsetOnAxis(ap=toku[:, s:s + 1],
                                                     axis=0),
                    ysb[tt][:], None, bounds_check=T + 127, oob_is_err=False)

        # ======== phase 6: ReduceScatter (overlaps shared FFN2) ========
        nc.gpsimd.collective_compute(
            "ReduceScatter", mybir.AluOpType.add,
            replica_groups=[list(range(N_CORES))],
            ins=[partial[0:T, :].opt()], outs=[rs_out[:].opt()])

        if debug:
            for r in range(T // 128):
                for hh in range(2):
                    dbt = stream.tile([128, 512], bf, tag="dbt", bufs=2,
                                      name=f"dbt{r}_{hh}")
                    nc.sync.dma_start(
                        dbt[:], partial[r * 128:(r + 1) * 128,
                                        hh * 512:hh * 512 + 512])
                    nc.sync.dma_start(
                        dbg_partial.ap()[r * 128:(r + 1) * 128,
                                         hh * 512:hh * 512 + 512], dbt[:])

        # ======== phase 7: shared FFN2 (resident sW2, no DMA while the
        # ReduceScatter owns the engines) + b2 combine + final add.
        # All 8 psum->SBUF copies are emitted BEFORE any RS-dependent add:
        # the DVE wait-queue is only 4 deep, so a blocked add would trap
        # later psum copies and stall the dh=1 matmuls on psum recycling.
        yshs = {}
        for dh in range(2):
            pys = [ppy.tile([128, 512], f32, tag="py", name=f"spys{dh}_{tt}")
                   for tt in range(4)]
            for m in range(MT):
                for tt in range(4):
                    nc.tensor.matmul(
                        pys[tt][:], lhsT=shT[m][:, tt * 128:(tt + 1) * 128],
                        rhs=sW2sb[:, m * D + dh * 512:m * D + dh * 512 + 512],
                        start=(m == 0), stop=False)
            for tt in range(4):
                nc.tensor.matmul(
                    pys[tt][:], lhsT=combT[0:9, tt * 128:(tt + 1) * 128],
                    rhs=b2psb[0:9, dh * 512:dh * 512 + 512],
                    start=False, stop=True)
                ysh = jtp.tile([128, 512], bf, tag="ysh", bufs=8,
                               name=f"ysh{dh}_{tt}")
                nc.vector.tensor_copy(ysh[:], pys[tt][:])
                yshs[(dh, tt)] = ysh
        for dh in range(2):
            for tt in range(4):
                rsb = ypool.tile([128, 512], bf, tag="rsb", bufs=2)
                nc.sync.dma_start(rsb[:], rs_out[tt * 128:(tt + 1) * 128,
                                                 dh * 512:dh * 512 + 512])
                outf = ypool.tile([128, 512], f32, tag="outf", bufs=1)
                nc.vector.tensor_tensor(outf[:], yshs[(dh, tt)][:], rsb[:],
                                        op=OP.add)
                nc.sync.dma_start(out_d.ap()[tt * 128:(tt + 1) * 128,
                                             dh * 512:dh * 512 + 512],
                                  outf[:])

    nc.compile()
    return nc


def _prep_inputs(hidden_states, Wg, W1, b1, W2, b2, sW1, sb1, sW2, sb2):
    """Host-side sharding/layout: per-core input dicts."""
    X = np.ascontiguousarray(hidden_states.reshape(T, D).astype(np.float32))
    Xbf = X.astype(BF16)
    tok_abs = np.arange(NJ)[None, :] * 128 + np.arange(128)[:, None]
    WgT = np.ascontiguousarray(
        Wg.T.reshape(KT, 128, E).transpose(1, 0, 2)
        .reshape(128, KT * E)).astype(np.float32)
    utri = np.triu(np.ones((128, 128), np.float32))
    sut = np.triu(np.ones((32, 32), np.float32), k=1)
    id128 = np.eye(128, dtype=np.float32)
    idbf = np.eye(128, dtype=np.float32).astype(BF16)

    sW1e = np.ascontiguousarray(
        sW1.reshape(KT, 128, MT, 128).transpose(1, 2, 0, 3)
        .reshape(128, MT * KT * 128)).astype(BF16)
    sW2e = np.ascontiguousarray(
        sW2.reshape(MT, 128, D).transpose(1, 0, 2).reshape(128, MT * D)
    ).astype(BF16)
    sb1e = np.ascontiguousarray(sb1.reshape(MT, 128).T).astype(np.float32)
    b2p9 = np.concatenate([b2, sb2[None, :]], axis=0).astype(BF16)

    in_maps = []
    for c in range(N_CORES):
        W1e = np.ascontiguousarray(
            W1[c].reshape(KT, 128, MT, 128).transpose(1, 2, 0, 3)
            .reshape(128, MT * KT * 128)).astype(BF16)
        W2e = np.ascontiguousarray(
            W2[c].reshape(MT, 128, D).transpose(1, 0, 2).reshape(128, MT * D)
        ).astype(BF16)
        b1e = np.ascontiguousarray(b1[c].reshape(MT, 128).T).astype(np.float32)
        myhot = np.zeros((128, E), np.float32)
        myhot[:, c] = 1.0
        # rotate token order so core c's home tokens come first (the
        # replicated gate + slotmap run in this order; token indices stay
        # absolute via tokf)
        Xrot = np.roll(X, -c * HOME, axis=0)
        XT32 = np.ascontiguousarray(
            Xrot.T.reshape(KT, 128, T).transpose(1, 0, 2).reshape(128, KT * T))
        tokf = ((tok_abs + c * HOME) % T).astype(np.float32)
        XhT = np.ascontiguousarray(
            X[c * HOME:(c + 1) * HOME].T.reshape(KT, 128, HOME)
            .transpose(1, 0, 2).reshape(128, KT * HOME))
        in_maps.append({
            "Xbf": Xbf, "XT32": XT32, "XhTbf": XhT.astype(BF16), "WgT": WgT,
            "W1e": W1e, "W2e": W2e, "b1e": b1e,
            "sW1e": sW1e, "sW2e": sW2e, "sb1e": sb1e, "b2p9": b2p9,
            "myhot": myhot, "tokf": tokf,
            "utri": utri, "sutri32": sut, "id128": id128, "idbf": idbf,
        })
    return in_maps


def kernel_run(inputs: dict, trace: bool = False, trace_cores=None,
               debug: bool = False):
    """Run the SPMD kernel; returns (full_output, BassKernelResults)."""
    key = ("nc", debug)
    if key not in _CACHE:
        _CACHE[key] = _build_nc(debug=debug)
    nc = _CACHE[key]
    in_maps = _prep_inputs(**{k: np.asarray(v) for k, v in inputs.items()})
    kw = {}
    if trace:
        kw = dict(trace=True,
                  trace_cores=trace_cores if trace_cores is not None else [0])
    res = run_bass_kernel_spmd(nc, in_maps, core_ids=list(range(N_CORES)), **kw)
    out = np.concatenate([res.results[c]["out"] for c in range(N_CORES)],
                         axis=0)
    bsz = inputs["hidden_states"].shape[0]
    return out.reshape(bsz, -1, D).astype(np.float32), res


def kernel(**inputs) -> np.ndarray:
    out, _ = kernel_run(inputs)
    return out
